# revision 8
# baseline (speedup 1.0000x reference)
"""Causal multi-head attention on 8 Trainium2 NeuronCores.

Sharding: data-parallel over batch (B=2) x tensor-parallel over heads
(16 heads -> 4 groups of 4). Core c handles batch c//4, head group c%4.
Each core computes q/k/v projections for its 4 heads, causal flash
attention, and a partial output projection (row slice of Wo); the host
sums the 4 partials per batch element.

All operands arrive pre-transposed from the host (numpy prep is free),
so the kernel runs zero PE transposes: xT/wqT/wkT/wvT/woT DMA straight
into SBUF in matmul-ready layout. Matmuls run in bf16 (fp32 PSUM
accumulation). The softmax row-sum is fused into the o^T = [v|1s]^T P^T
matmul via an appended ones column; normalization (broadcast rowsum via
K=1 matmul reading partition 64, fast-approx reciprocal, divide) stays
in fp32.

The whole kernel is one software-pipelined stream: projection chains
for s-chunk sc+1 and v-projections are queued as background PE work and
drain between the QK->exp slots of attention q-chunk qc=sc, alongside
the AV matmuls, normalization epilogues, and output projections.  The
queue drain is paced so the PE never outruns the ACT-engine exp chain
(psS double buffering) and never idles.
"""

import numpy as np
import ml_dtypes

import concourse.bacc as bacc
import concourse.bass as bass
import concourse.tile as tile
from concourse import bass_utils, mybir

B, S, D, H = 2, 2048, 1024, 16
DK = 64
NH = 4                 # heads per core
E = NH * DK            # 256: per-core head-dim slice
SCALE = 1.0 / 8.0      # 1/sqrt(DK)

F32 = mybir.dt.float32
F32R = mybir.dt.float32r
BF16 = mybir.dt.bfloat16

QC = 512               # q-chunk (columns per attention tile)
NQC = S // QC          # 4
NKB = S // 128         # 16 k-blocks


def _emit(tc, nc, xT_d, wqT_d, wkT_d, wvT_d, woT_d, yT_d, mask_d, ones_d):
    const = tc.alloc_tile_pool(name="const", bufs=1)
    perm = tc.alloc_tile_pool(name="perm", bufs=1)

    # constants first (small): mask for the causal diagonal, ones for the
    # v ones-column and the rowsum broadcast lhsT
    mask = const.tile([128, 128], BF16)
    nc.scalar.dma_start(out=mask, in_=mask_d)
    ones_f32 = const.tile([128, 64], F32)
    nc.scalar.dma_start(out=ones_f32, in_=ones_d)
    ones128 = const.tile([128, 64], F32R)
    nc.vector.tensor_copy(ones128, ones_f32)

    # big operands, all pre-transposed by the host. x chunks stream on the
    # sync HWDGE queue while weights go down the scalar queue in parallel
    # (each queue runs near HBM line rate; splitting halves the time to
    # the first projection's operands).
    xT = [perm.tile([128, 8, QC], BF16, name=f"xT{sc}") for sc in range(4)]
    wqT = perm.tile([128, 8, E], BF16)    # wqT[p, dc, e] = wq[e, dc*128+p]
    wkT = perm.tile([128, 8, E], BF16)
    wvT = perm.tile([128, 8, E], BF16)
    woT = perm.tile([128, 2, D], BF16)    # woT[p, ec, o] = wo[o, ec*128+p]
    qT = perm.tile([128, 2, S], BF16)     # qT[p, ec, s] = q[s, ec*128+p]
    kT = perm.tile([128, 2, S], BF16)
    v_sb = perm.tile([128, NKB, NH, DK + 1], BF16)  # [.., 64] = ones column

    for sc in range(4):
        nc.sync.dma_start(out=xT[sc],
                          in_=xT_d[:, :, sc * QC:(sc + 1) * QC])
    nc.scalar.dma_start(out=wqT, in_=wqT_d)
    nc.scalar.dma_start(out=wkT, in_=wkT_d)
    nc.scalar.dma_start(out=wvT, in_=wvT_d)
    nc.scalar.dma_start(out=woT, in_=woT_d)

    ncopy = [0]

    def copy(dst, src):
        # psum->sbuf projection copies: mostly DVE; every 3rd on ACT
        # (ACT's exp load ramps up only in the later q-chunks)
        if ncopy[0] % 3 != 2:
            nc.vector.tensor_copy(dst, src)
        else:
            nc.scalar.copy(dst, src)
        ncopy[0] += 1

    work = tc.alloc_tile_pool(name="work", bufs=3)
    small = tc.alloc_tile_pool(name="small", bufs=2)

    with tc.tile_pool(name="psS", bufs=1, space="PSUM") as ps_S, \
         tc.tile_pool(name="psO", bufs=1, space="PSUM") as ps_o, \
         tc.tile_pool(name="psY", bufs=1, space="PSUM") as ps_y:

        # ---- background PE work units ----
        def make_proj(w_t, outT, ec, sc):
            def u():
                ps = ps_y.tile([128, QC], F32, tag="y", bufs=2, name="psp")
                for dc in range(8):
                    nc.tensor.matmul(
                        ps,
                        lhsT=w_t[:, dc, ec * 128:(ec + 1) * 128],
                        rhs=xT[sc][:, dc, :],
                        start=(dc == 0),
                        stop=(dc == 7),
                    )
                copy(outT[:, ec, sc * QC:(sc + 1) * QC], ps)
            return u

        def make_vproj(sblk):
            def u():
                ps = ps_y.tile([128, E], F32, tag="y", bufs=2, name="psv")
                for dc in range(8):
                    nc.tensor.matmul(
                        ps,
                        lhsT=xT[sblk // 4][:, dc,
                                           (sblk % 4) * 128:(sblk % 4 + 1) * 128],
                        rhs=wvT[:, dc, :],
                        start=(dc == 0),
                        stop=(dc == 7),
                    )
                # scatter 4 heads into [.., l, 0:64]
                sap = bass.AP(
                    tensor=ps.tensor, offset=ps.offset,
                    ap=[ps.ap[0], [DK, NH], [1, DK]],
                )
                nc.vector.tensor_copy(v_sb[:, sblk, :, 0:DK], sap)
            return u

        def make_av(po_box, pts, kb, hp, kmax, qc):
            def av():
                if po_box[0] is None:
                    po_box[0] = (
                        ps_o.tile([DK + 1, QC], F32, tag="o", bufs=2, name="poA"),
                        ps_o.tile([DK + 1, QC], F32, tag="o", bufs=2, name="poB"),
                    )
                poA, poB = po_box[0]
                pT, cs = pts[kb]
                for hi, po in ((0, poA), (1, poB)):
                    nc.tensor.matmul(
                        po[:, cs:512],
                        lhsT=v_sb[:, kb, 2 * hp + hi, :],
                        rhs=pT[:, hi, cs:512],
                        start=(kb == 0),
                        stop=(kb == kmax - 1),
                    )
            return av

        def make_epilogue(po_box, oT, hp):
            def epi():
                poA, poB = po_box[0]
                # free the psum banks quickly with one copy per head
                oA_sb = small.tile([DK + 1, QC], F32R, tag="osb", bufs=4)
                oB_sb = small.tile([DK + 1, QC], F32R, tag="osb", bufs=4)
                nc.vector.tensor_copy(oA_sb, poA)
                nc.vector.tensor_copy(oB_sb, poB)
                for hi, o_sb in ((0, oA_sb), (1, oB_sb)):
                    # broadcast rowsum (row 64) to 64 partitions via K=1
                    # matmul reading partition 64 (row group 64)
                    ps_bc = ps_y.tile([64, QC], F32, tag="y", bufs=2, name="psbc")
                    nc.tensor.matmul(
                        ps_bc,
                        lhsT=ones128[64:65, :],
                        rhs=o_sb[DK:DK + 1, :],
                        start=True,
                        stop=True,
                    )
                    rec = small.tile([64, QC], F32, tag="rec", bufs=2)
                    nc.vector.reciprocal_approx_fast(rec, ps_bc)
                    if hi == 0:
                        nc.vector.tensor_mul(oT[0:64, hp, :], o_sb[0:DK, :], rec)
                    else:
                        tmpB = small.tile([64, QC], BF16, tag="tmpB", bufs=2)
                        nc.vector.tensor_mul(tmpB, o_sb[0:DK, :], rec)
                        # partition shift 0-63 -> 64-127 via sbuf->sbuf DMA
                        nc.sync.dma_start(out=oT[64:128, hp, :], in_=tmpB)
            return epi

        def make_out_proj(qc, oT):
            units = []
            for dc in range(8):
                def u(dc=dc, qc=qc, oT=oT):
                    psy = ps_y.tile([128, QC], F32, tag="y", bufs=2, name="psy")
                    for ec in range(2):
                        nc.tensor.matmul(
                            psy,
                            lhsT=woT[:, ec, dc * 128:(dc + 1) * 128],
                            rhs=oT[:, ec, :],
                            start=(ec == 0),
                            stop=(ec == 1),
                        )
                    y_sb = work.tile([128, QC], BF16, tag="ysb", bufs=3)
                    nc.vector.tensor_copy(y_sb, psy)
                    nc.sync.dma_start(
                        out=yT_d[:, dc, qc * QC:(qc + 1) * QC],
                        in_=y_sb,
                    )
                units.append(u)
            return units

        # ---- the single pipelined stream ----
        # workq holds (weight, closure, min_slot): min_slot enforces a lag
        # so a unit whose inputs were just emitted (exp -> AV) never
        # head-of-line-blocks the PE FIFO.
        workq = []
        slot_i = [0]
        drained = [0]
        enqueued_n = [0]
        popped_n = [0]

        def enq(w, u, lag=0):
            workq.append((w, u, slot_i[0] + lag))
            enqueued_n[0] += 1

        def drain(budget):
            while workq and budget > 0 and workq[0][2] <= slot_i[0]:
                w, u, _ = workq.pop(0)
                u()
                drained[0] += w
                popped_n[0] += 1
                budget -= w

        def force_drain_to(n):
            # pop (ignoring lag) until n items have been popped in total:
            # used at chunk boundaries so the projections a chunk's QK
            # depends on are already in the PE FIFO ahead of it
            while workq and popped_n[0] < n:
                w, u, _ = workq.pop(0)
                u()
                drained[0] += w
                popped_n[0] += 1

        # weights ~ PE-time in ~200ns units
        W_PROJ, W_VPROJ, W_AV, W_EPI, W_OP = 9, 5, 2, 3, 2

        # projections for sc=0 and the first v blocks run inline (nothing
        # to interleave with yet -- the PE is waiting on the first DMAs)
        for w_t, outT in ((wqT, qT), (wkT, kT)):
            for ec in range(2):
                make_proj(w_t, outT, ec, 0)()
        for sblk in range(4):
            make_vproj(sblk)()

        # ones column of v (written once; strided 3D AP)
        ones_ap = bass.AP(
            tensor=v_sb.tensor,
            offset=v_sb.offset + DK,
            ap=[v_sb.ap[0], [NH * (DK + 1), NKB], [DK + 1, NH]],
        )
        src64 = bass.AP(
            tensor=ones_f32.tensor, offset=ones_f32.offset,
            ap=[ones_f32.ap[0], [4, NKB], [1, NH]],
        )
        nc.vector.tensor_copy(ones_ap, src64)

        TOTAL_SLOTS = 2 * sum(4 * (q + 1) for q in range(NQC))       # 80
        W_TOTAL = (12 * W_PROJ + 12 * W_VPROJ + 80 * W_AV
                   + 8 * W_EPI + 32 * W_OP)

        batch_end = [0, 0, 0, 0]  # popped-count needed before chunk qc

        for qc in range(NQC):
            force_drain_to(batch_end[qc])
            # queue next chunk's projections + the v blocks attention
            # qc+1 will need; FIFO order keeps them ahead of qc's AV work
            if qc + 1 < NQC:
                for w_t, outT in ((wqT, qT), (wkT, kT)):
                    for ec in range(2):
                        enq(W_PROJ, make_proj(w_t, outT, ec, qc + 1))
                for sblk in range(4 * (qc + 1), 4 * (qc + 2)):
                    enq(W_VPROJ, make_vproj(sblk))
                batch_end[qc + 1] = enqueued_n[0]

            oT = work.tile([128, 2, QC], BF16, tag="oT", bufs=2)
            kmax = 4 * (qc + 1)
            for hp in range(2):
                pts = {}
                po_box = [None]
                for kb in range(kmax):
                    # S^T = k q^T, 2-head row-tiled pair, causally narrowed
                    cs = max(0, kb * 128 - qc * QC)
                    psS = ps_S.tile([128, 2, 512], F32, tag="S", bufs=2)
                    for hi in range(2):
                        nc.tensor.matmul(
                            psS[:, hi, cs:512],
                            lhsT=kT[hi * 64:(hi + 1) * 64, hp,
                                    kb * 128:(kb + 1) * 128],
                            rhs=qT[hi * 64:(hi + 1) * 64, hp,
                                   qc * QC + cs:(qc + 1) * QC],
                            start=True,
                            stop=True,
                        )
                    pT = work.tile([128, 2, 512], BF16, tag="pT", bufs=24)
                    pts[kb] = (pT, cs)
                    nc.scalar.activation(
                        pT[:, :, cs:512],
                        psS[:, :, cs:512],
                        mybir.ActivationFunctionType.Exp,
                        scale=SCALE,
                    )
                    if kb >= 4 * qc:  # diagonal band: zero the upper triangle
                        mask2 = bass.AP(
                            tensor=mask.tensor, offset=mask.offset,
                            ap=[mask.ap[0], [0, 2], mask.ap[1]],
                        )
                        nc.vector.tensor_mul(
                            pT[:, :, cs:cs + 128],
                            pT[:, :, cs:cs + 128],
                            mask2,
                        )
                    # AV for this block enters the queue 2 slots behind so
                    # its exp has drained by the time the PE reaches it
                    enq(W_AV, make_av(po_box, pts, kb, hp, kmax, qc), lag=2)
                    slot_i[0] += 1
                    target = (W_TOTAL * slot_i[0]) // TOTAL_SLOTS
                    drain(min(10, target - drained[0]))
                enq(W_EPI, make_epilogue(po_box, oT, hp), lag=2)
                if hp == 1:
                    for u in make_out_proj(qc, oT):
                        enq(W_OP, u, lag=3)
        slot_i[0] += 10
        drain(10**9)

    for p in [small, work, perm, const]:
        p.release()


_CACHE = {}


def _build():
    if "nc" in _CACHE:
        return _CACHE["nc"]
    nc = bacc.Bacc("TRN2", target_bir_lowering=False, debug=False, num_devices=8)
    xT_d = nc.dram_tensor("xT", [128, 8, S], BF16, kind="ExternalInput").ap()
    wqT_d = nc.dram_tensor("wqT", [128, 8, E], BF16, kind="ExternalInput").ap()
    wkT_d = nc.dram_tensor("wkT", [128, 8, E], BF16, kind="ExternalInput").ap()
    wvT_d = nc.dram_tensor("wvT", [128, 8, E], BF16, kind="ExternalInput").ap()
    woT_d = nc.dram_tensor("woT", [128, 2, D], BF16, kind="ExternalInput").ap()
    yT_d = nc.dram_tensor("yT", [128, 8, S], BF16, kind="ExternalOutput").ap()
    mask_d = nc.dram_tensor("maskc", [128, 128], BF16, kind="ExternalInput").ap()
    ones_d = nc.dram_tensor("onesc", [128, 64], F32, kind="ExternalInput").ap()
    with tile.TileContext(nc) as tc:
        _emit(tc, nc, xT_d, wqT_d, wkT_d, wvT_d, woT_d, yT_d, mask_d, ones_d)
    nc.compile()
    _CACHE["nc"] = nc
    return nc


_r = np.arange(128)
_MASK = np.where(_r[:, None] <= _r[None, :], 1.0, 0.0).astype(ml_dtypes.bfloat16)
_ONES = np.ones((128, 64), dtype=np.float32)

LAST_RESULT = None


def _to_pdc(a):
    """[R, C] -> [128, R//128, C]: partition-major DMA layout."""
    r, c = a.shape
    return np.ascontiguousarray(a.reshape(r // 128, 128, c).transpose(1, 0, 2))


def kernel(x, wq, wk, wv, wo):
    global LAST_RESULT
    nc = _build()
    bf = ml_dtypes.bfloat16
    x = np.asarray(x, dtype=np.float32)
    wq = np.asarray(wq, dtype=np.float32)
    wk = np.asarray(wk, dtype=np.float32)
    wv = np.asarray(wv, dtype=np.float32)
    wo = np.asarray(wo, dtype=np.float32)

    in_maps = []
    xT_b = [_to_pdc(x[b].T.astype(bf)) for b in range(B)]
    for c in range(8):
        b, g = c // 4, c % 4
        rows = slice(g * E, (g + 1) * E)
        in_maps.append({
            "xT": xT_b[b],
            "wqT": _to_pdc(wq[rows].T.astype(bf)),
            "wkT": _to_pdc(wk[rows].T.astype(bf)),
            "wvT": _to_pdc(wv[rows].T.astype(bf)),
            "woT": _to_pdc(wo[:, rows].T.astype(bf)),
            "maskc": _MASK,
            "onesc": _ONES,
        })

    res = bass_utils.run_bass_kernel_spmd(nc, in_maps, core_ids=list(range(8)))
    LAST_RESULT = res

    y = np.empty((B, S, D), dtype=np.float32)
    for b in range(B):
        acc = res.results[4 * b]["yT"].astype(np.float32)
        for g in range(1, 4):
            acc += res.results[4 * b + g]["yT"]
        # yT layout [128, 8, S]: [p, dc, s] = y[s, dc*128+p]
        y[b] = acc.transpose(2, 1, 0).reshape(S, D)
    return y


# revision 18
# speedup vs baseline: 1.0011x; 1.0011x over previous
"""Causal multi-head attention on 8 Trainium2 NeuronCores.

Sharding: data-parallel over batch (B=2) x tensor-parallel over heads
(16 heads -> 4 groups of 4). Core c handles batch c//4, head group c%4.
Each core computes q/k/v projections for its 4 heads, causal flash
attention, and a partial output projection (row slice of Wo); the host
sums the 4 partials per batch element.

All operands arrive pre-transposed from the host (numpy prep is free),
so the kernel runs zero PE transposes: xT/wqT/wkT/wvT/woT DMA straight
into SBUF in matmul-ready layout. Matmuls run in bf16 (fp32 PSUM
accumulation). The softmax row-sum is fused into the o^T = [v|1s]^T P^T
matmul via an appended ones column; normalization (broadcast rowsum via
K=1 matmul reading partition 64, fast-approx reciprocal, divide) stays
in fp32.

The whole kernel is one software-pipelined stream: projection chains
for s-chunk sc+1 and v-projections are queued as background PE work and
drain between the QK->exp slots of attention q-chunk qc=sc, alongside
the AV matmuls, normalization epilogues, and output projections.  The
queue drain is paced so the PE never outruns the ACT-engine exp chain
(psS double buffering) and never idles.
"""

import numpy as np
import ml_dtypes

import concourse.bacc as bacc
import concourse.bass as bass
import concourse.tile as tile
from concourse import bass_utils, mybir

B, S, D, H = 2, 2048, 1024, 16
DK = 64
NH = 4                 # heads per core
E = NH * DK            # 256: per-core head-dim slice
SCALE = 1.0 / 8.0      # 1/sqrt(DK)

F32 = mybir.dt.float32
F32R = mybir.dt.float32r
BF16 = mybir.dt.bfloat16

QC = 512               # q-chunk (columns per attention tile)
NQC = S // QC          # 4
NKB = S // 128         # 16 k-blocks


def _emit(tc, nc, xT_d, wqT_d, wkT_d, wvT_d, woT_d, yT_d, mask_d, ones_d):
    const = tc.alloc_tile_pool(name="const", bufs=1)
    perm = tc.alloc_tile_pool(name="perm", bufs=1)

    # constants first (small): mask for the causal diagonal, ones for the
    # v ones-column and the rowsum broadcast lhsT
    mask = const.tile([128, 128], BF16)
    nc.scalar.dma_start(out=mask, in_=mask_d)
    ones_f32 = const.tile([128, 64], F32)
    nc.scalar.dma_start(out=ones_f32, in_=ones_d)
    ones128 = const.tile([128, 64], F32R)
    nc.vector.tensor_copy(ones128, ones_f32)

    # big operands, all pre-transposed by the host. x chunks stream on the
    # sync HWDGE queue while weights go down the scalar queue in parallel
    # (each queue runs near HBM line rate; splitting halves the time to
    # the first projection's operands).
    xT = [perm.tile([128, 8, QC], BF16, name=f"xT{sc}") for sc in range(4)]
    wqT = perm.tile([128, 8, E], BF16)    # wqT[p, dc, e] = wq[e, dc*128+p]
    wkT = perm.tile([128, 8, E], BF16)
    wvT = perm.tile([128, 8, E], BF16)
    woT = perm.tile([128, 2, D], BF16)    # woT[p, ec, o] = wo[o, ec*128+p]
    qT = perm.tile([128, 2, S], BF16)     # qT[p, ec, s] = q[s, ec*128+p]
    kT = perm.tile([128, 2, S], BF16)
    v_sb = perm.tile([128, NKB, NH, DK + 1], BF16)  # [.., 64] = ones column

    # sync queue (starts earliest): the critical chain for the first
    # projection units, finest pieces first so MMs can begin on dc 0-3
    # while dc 4-7 are still in flight (tile deps are region-granular)
    nc.sync.dma_start(out=wqT[:, 0:4, :], in_=wqT_d[:, 0:4, :])
    nc.sync.dma_start(out=xT[0][:, 0:4, :], in_=xT_d[:, 0:4, 0:QC])
    nc.sync.dma_start(out=wqT[:, 4:8, :], in_=wqT_d[:, 4:8, :])
    nc.sync.dma_start(out=xT[0][:, 4:8, :], in_=xT_d[:, 4:8, 0:QC])
    nc.sync.dma_start(out=wkT, in_=wkT_d)
    nc.sync.dma_start(out=xT[1], in_=xT_d[:, :, QC:2 * QC])
    # scalar queue: everything needed later, in need order
    nc.scalar.dma_start(out=wvT, in_=wvT_d)
    nc.scalar.dma_start(out=xT[2], in_=xT_d[:, :, 2 * QC:3 * QC])
    nc.scalar.dma_start(out=xT[3], in_=xT_d[:, :, 3 * QC:4 * QC])
    nc.scalar.dma_start(out=woT, in_=woT_d)

    ncopy = [0]

    def copy(dst, src):
        # psum->sbuf projection copies: mostly DVE; every 3rd on ACT
        # (ACT's exp load ramps up only in the later q-chunks)
        if ncopy[0] % 3 != 2:
            nc.vector.tensor_copy(dst, src)
        else:
            nc.scalar.copy(dst, src)
        ncopy[0] += 1

    work = tc.alloc_tile_pool(name="work", bufs=3)
    small = tc.alloc_tile_pool(name="small", bufs=2)

    with tc.tile_pool(name="psS", bufs=1, space="PSUM") as ps_S, \
         tc.tile_pool(name="psO", bufs=1, space="PSUM") as ps_o, \
         tc.tile_pool(name="psY", bufs=1, space="PSUM") as ps_y:

        # ---- background PE work units ----
        def make_proj(w_t, outT, ec, sc):
            def u():
                ps = ps_y.tile([128, QC], F32, tag="y", bufs=2, name="psp")
                for dc in range(8):
                    nc.tensor.matmul(
                        ps,
                        lhsT=w_t[:, dc, ec * 128:(ec + 1) * 128],
                        rhs=xT[sc][:, dc, :],
                        start=(dc == 0),
                        stop=(dc == 7),
                    )
                copy(outT[:, ec, sc * QC:(sc + 1) * QC], ps)
            return u

        def make_vproj(sblk):
            def u():
                ps = ps_y.tile([128, E], F32, tag="y", bufs=2, name="psv")
                for dc in range(8):
                    nc.tensor.matmul(
                        ps,
                        lhsT=xT[sblk // 4][:, dc,
                                           (sblk % 4) * 128:(sblk % 4 + 1) * 128],
                        rhs=wvT[:, dc, :],
                        start=(dc == 0),
                        stop=(dc == 7),
                    )
                # scatter 4 heads into [.., l, 0:64]
                sap = bass.AP(
                    tensor=ps.tensor, offset=ps.offset,
                    ap=[ps.ap[0], [DK, NH], [1, DK]],
                )
                nc.vector.tensor_copy(v_sb[:, sblk, :, 0:DK], sap)
            return u

        def make_av(po_box, pts, kb, hp, kmax, qc):
            def av():
                if po_box[0] is None:
                    po_box[0] = (
                        ps_o.tile([DK + 1, QC], F32, tag="o", bufs=2, name="poA"),
                        ps_o.tile([DK + 1, QC], F32, tag="o", bufs=2, name="poB"),
                    )
                poA, poB = po_box[0]
                pT, cs = pts[kb]
                for hi, po in ((0, poA), (1, poB)):
                    nc.tensor.matmul(
                        po[:, cs:512],
                        lhsT=v_sb[:, kb, 2 * hp + hi, :],
                        rhs=pT[:, hi, cs:512],
                        start=(kb == 0),
                        stop=(kb == kmax - 1),
                    )
            return av

        def make_epilogue(po_box, oT, hp):
            def epi():
                poA, poB = po_box[0]
                # free the psum banks quickly with one copy per head
                oA_sb = small.tile([DK + 1, QC], F32R, tag="osb", bufs=4)
                oB_sb = small.tile([DK + 1, QC], F32R, tag="osb", bufs=4)
                nc.vector.tensor_copy(oA_sb, poA)
                nc.vector.tensor_copy(oB_sb, poB)
                for hi, o_sb in ((0, oA_sb), (1, oB_sb)):
                    # broadcast rowsum (row 64) to 64 partitions via K=1
                    # matmul reading partition 64 (row group 64)
                    ps_bc = ps_y.tile([64, QC], F32, tag="y", bufs=2, name="psbc")
                    nc.tensor.matmul(
                        ps_bc,
                        lhsT=ones128[64:65, :],
                        rhs=o_sb[DK:DK + 1, :],
                        start=True,
                        stop=True,
                    )
                    rec = small.tile([64, QC], F32, tag="rec", bufs=2)
                    nc.vector.reciprocal_approx_fast(rec, ps_bc)
                    if hi == 0:
                        nc.vector.tensor_mul(oT[0:64, hp, :], o_sb[0:DK, :], rec)
                    else:
                        tmpB = small.tile([64, QC], BF16, tag="tmpB", bufs=2)
                        nc.vector.tensor_mul(tmpB, o_sb[0:DK, :], rec)
                        # partition shift 0-63 -> 64-127 via sbuf->sbuf DMA
                        nc.sync.dma_start(out=oT[64:128, hp, :], in_=tmpB)
            return epi

        def make_out_proj(qc, oT):
            # one [128, 8, QC] staging tile per chunk: 8 psum copies land
            # in it, then a single 1MB store (fewer DMAs + semaphores)
            y_sb = work.tile([128, 8, QC], BF16, tag="ysb", bufs=2)
            units = []
            for dc in range(8):
                def u(dc=dc, qc=qc, oT=oT, y_sb=y_sb):
                    psy = ps_y.tile([128, QC], F32, tag="y", bufs=2, name="psy")
                    for ec in range(2):
                        nc.tensor.matmul(
                            psy,
                            lhsT=woT[:, ec, dc * 128:(dc + 1) * 128],
                            rhs=oT[:, ec, :],
                            start=(ec == 0),
                            stop=(ec == 1),
                        )
                    if qc == NQC - 1:
                        # last chunk: ACT's exp work is done, DVE is the
                        # tail bottleneck -- move these copies to ACT
                        nc.scalar.copy(y_sb[:, dc, :], psy)
                    else:
                        nc.vector.tensor_copy(y_sb[:, dc, :], psy)
                    if dc == 7:
                        nc.sync.dma_start(
                            out=yT_d[:, :, qc * QC:(qc + 1) * QC],
                            in_=y_sb,
                        )
                units.append(u)
            return units

        # ---- the single pipelined stream ----
        # workq holds (weight, closure, min_slot): min_slot enforces a lag
        # so a unit whose inputs were just emitted (exp -> AV) never
        # head-of-line-blocks the PE FIFO.
        workq = []
        slot_i = [0]
        drained = [0]
        enqueued_n = [0]
        popped_n = [0]

        def enq(w, u, lag=0):
            workq.append((w, u, slot_i[0] + lag))
            enqueued_n[0] += 1

        def drain(budget):
            while workq and budget > 0 and workq[0][2] <= slot_i[0]:
                w, u, _ = workq.pop(0)
                u()
                drained[0] += w
                popped_n[0] += 1
                budget -= w

        def force_drain_to(n):
            # pop (ignoring lag) until n items have been popped in total:
            # used at chunk boundaries so the projections a chunk's QK
            # depends on are already in the PE FIFO ahead of it
            while workq and popped_n[0] < n:
                w, u, _ = workq.pop(0)
                u()
                drained[0] += w
                popped_n[0] += 1

        # weights ~ PE-time in ~200ns units
        W_PROJ, W_VPROJ, W_AV, W_EPI, W_OP = 9, 5, 2, 3, 2

        # projections for sc=0 and the first v blocks run inline (nothing
        # to interleave with yet -- the PE is waiting on the first DMAs)
        for w_t, outT in ((wqT, qT), (wkT, kT)):
            for ec in range(2):
                make_proj(w_t, outT, ec, 0)()
        for sblk in range(4):
            make_vproj(sblk)()

        # ones column of v (written once; strided 3D AP)
        ones_ap = bass.AP(
            tensor=v_sb.tensor,
            offset=v_sb.offset + DK,
            ap=[v_sb.ap[0], [NH * (DK + 1), NKB], [DK + 1, NH]],
        )
        src64 = bass.AP(
            tensor=ones_f32.tensor, offset=ones_f32.offset,
            ap=[ones_f32.ap[0], [4, NKB], [1, NH]],
        )
        nc.vector.tensor_copy(ones_ap, src64)

        TOTAL_SLOTS = 2 * sum(4 * (q + 1) for q in range(NQC))       # 80
        W_TOTAL = (12 * W_PROJ + 12 * W_VPROJ + 80 * W_AV
                   + 8 * W_EPI + 32 * W_OP)

        batch_end = [0, 0, 0, 0]  # popped-count needed before chunk qc

        for qc in range(NQC):
            force_drain_to(batch_end[qc])
            # queue next chunk's projections + the v blocks attention
            # qc+1 will need; FIFO order keeps them ahead of qc's AV work
            if qc + 1 < NQC:
                for w_t, outT in ((wqT, qT), (wkT, kT)):
                    for ec in range(2):
                        enq(W_PROJ, make_proj(w_t, outT, ec, qc + 1))
                for sblk in range(4 * (qc + 1), 4 * (qc + 2)):
                    enq(W_VPROJ, make_vproj(sblk))
                batch_end[qc + 1] = enqueued_n[0]

            oT = work.tile([128, 2, QC], BF16, tag="oT", bufs=2)
            kmax = 4 * (qc + 1)
            for hp in range(2):
                pts = {}
                po_box = [None]
                for kb in range(kmax):
                    # S^T = k q^T, 2-head row-tiled pair, causally narrowed
                    cs = max(0, kb * 128 - qc * QC)
                    psS = ps_S.tile([128, 2, 512], F32, tag="S", bufs=2)
                    for hi in range(2):
                        nc.tensor.matmul(
                            psS[:, hi, cs:512],
                            lhsT=kT[hi * 64:(hi + 1) * 64, hp,
                                    kb * 128:(kb + 1) * 128],
                            rhs=qT[hi * 64:(hi + 1) * 64, hp,
                                   qc * QC + cs:(qc + 1) * QC],
                            start=True,
                            stop=True,
                        )
                    pT = work.tile([128, 2, 512], BF16, tag="pT", bufs=24)
                    pts[kb] = (pT, cs)
                    nc.scalar.activation(
                        pT[:, :, cs:512],
                        psS[:, :, cs:512],
                        mybir.ActivationFunctionType.Exp,
                        scale=SCALE,
                    )
                    if kb >= 4 * qc:  # diagonal band: zero the upper triangle
                        mask2 = bass.AP(
                            tensor=mask.tensor, offset=mask.offset,
                            ap=[mask.ap[0], [0, 2], mask.ap[1]],
                        )
                        nc.vector.tensor_mul(
                            pT[:, :, cs:cs + 128],
                            pT[:, :, cs:cs + 128],
                            mask2,
                        )
                    # AV for this block enters the queue 2 slots behind so
                    # its exp has drained by the time the PE reaches it
                    enq(W_AV, make_av(po_box, pts, kb, hp, kmax, qc), lag=2)
                    slot_i[0] += 1
                    target = (W_TOTAL * slot_i[0]) // TOTAL_SLOTS
                    drain(min(10, target - drained[0]))
                enq(W_EPI, make_epilogue(po_box, oT, hp), lag=2)
                if hp == 1:
                    for u in make_out_proj(qc, oT):
                        enq(W_OP, u, lag=3)
        slot_i[0] += 10
        drain(10**9)

    for p in [small, work, perm, const]:
        p.release()


_CACHE = {}


def _build():
    if "nc" in _CACHE:
        return _CACHE["nc"]
    nc = bacc.Bacc("TRN2", target_bir_lowering=False, debug=False, num_devices=8)
    xT_d = nc.dram_tensor("xT", [128, 8, S], BF16, kind="ExternalInput").ap()
    wqT_d = nc.dram_tensor("wqT", [128, 8, E], BF16, kind="ExternalInput").ap()
    wkT_d = nc.dram_tensor("wkT", [128, 8, E], BF16, kind="ExternalInput").ap()
    wvT_d = nc.dram_tensor("wvT", [128, 8, E], BF16, kind="ExternalInput").ap()
    woT_d = nc.dram_tensor("woT", [128, 2, D], BF16, kind="ExternalInput").ap()
    yT_d = nc.dram_tensor("yT", [128, 8, S], BF16, kind="ExternalOutput").ap()
    mask_d = nc.dram_tensor("maskc", [128, 128], BF16, kind="ExternalInput").ap()
    ones_d = nc.dram_tensor("onesc", [128, 64], F32, kind="ExternalInput").ap()
    with tile.TileContext(nc) as tc:
        _emit(tc, nc, xT_d, wqT_d, wkT_d, wvT_d, woT_d, yT_d, mask_d, ones_d)
    nc.compile()
    _CACHE["nc"] = nc
    return nc


_r = np.arange(128)
_MASK = np.where(_r[:, None] <= _r[None, :], 1.0, 0.0).astype(ml_dtypes.bfloat16)
_ONES = np.ones((128, 64), dtype=np.float32)

LAST_RESULT = None


def _to_pdc(a):
    """[R, C] -> [128, R//128, C]: partition-major DMA layout."""
    r, c = a.shape
    return np.ascontiguousarray(a.reshape(r // 128, 128, c).transpose(1, 0, 2))


def kernel(x, wq, wk, wv, wo):
    global LAST_RESULT
    nc = _build()
    bf = ml_dtypes.bfloat16
    x = np.asarray(x, dtype=np.float32)
    wq = np.asarray(wq, dtype=np.float32)
    wk = np.asarray(wk, dtype=np.float32)
    wv = np.asarray(wv, dtype=np.float32)
    wo = np.asarray(wo, dtype=np.float32)

    in_maps = []
    xT_b = [_to_pdc(x[b].T.astype(bf)) for b in range(B)]
    for c in range(8):
        b, g = c // 4, c % 4
        rows = slice(g * E, (g + 1) * E)
        in_maps.append({
            "xT": xT_b[b],
            "wqT": _to_pdc(wq[rows].T.astype(bf)),
            "wkT": _to_pdc(wk[rows].T.astype(bf)),
            "wvT": _to_pdc(wv[rows].T.astype(bf)),
            "woT": _to_pdc(wo[:, rows].T.astype(bf)),
            "maskc": _MASK,
            "onesc": _ONES,
        })

    res = bass_utils.run_bass_kernel_spmd(nc, in_maps, core_ids=list(range(8)))
    LAST_RESULT = res

    y = np.empty((B, S, D), dtype=np.float32)
    for b in range(B):
        acc = res.results[4 * b]["yT"].astype(np.float32)
        for g in range(1, 4):
            acc += res.results[4 * b + g]["yT"]
        # yT layout [128, 8, S]: [p, dc, s] = y[s, dc*128+p]
        y[b] = acc.transpose(2, 1, 0).reshape(S, D)
    return y
